# revision 8
# baseline (speedup 1.0000x reference)
"""LRU single-step kernel for 8x TRN2 NeuronCores (Bass/Tile).

Math (per batch row b, hidden h):
  out_re[b,h] = lam_re[h]*h_re[b,h] - lam_im[h]*h_im[b,h] + (x @ (scale*B_real).T)[b,h]
  out_im[b,h] = lam_im[h]*h_re[b,h] + lam_re[h]*h_im[b,h] + (x @ (scale*B_img ).T)[b,h]

Strategy: data-parallel over the batch axis (8 shards of 32768 rows). Device
IO: x/weights/outputs in bf16, h_re/h_im (and the Lambda weight block) in
fp8e4m3 — numpy-simulated end-to-end rel-L2 err 0.0089, under the 2e-2 gate
with 2.2x margin. f32 accumulation in PSUM. Transposed layout: hidden on
partitions, batch on the free axis.

Complex-interleave trick: h_re/h_im for 64 hidden dims are interleaved on the
128 partitions (even partition 2u = h_re[64q+u], odd = h_im[64q+u]). A single
128x128 stationary matrix Wh_q then computes BOTH Lambda terms in one matmul:
out partitions 0:64 get lam_re*h_re - lam_im*h_im, partitions 64:128 get
lam_im*h_re + lam_re*h_im. The input projection accumulates into the same
PSUM tile via a column-interleaved weight block W_q = [w_re_q | w_im_q], so
each (q, col-block) PSUM tile takes exactly 2 matmuls:

  psum_q[m, n] = W_q[i, m].T @ x_t[i, n] + Wh_q[k, m].T @ h2_q[k, n]

This is ~2/3 less PE work than separate diag-matmul accumulation and keeps
ACT/DVE down to PSUM->SBUF bf16 copies; the kernel is DMA-bound. Host-side
prep (shard, transpose, interleave, bf16 cast, tiny param math) is numpy.

PE Matmult instructions only have one sync-wait slot in codegen, so waits are
absorbed before real matmuls run:
  - per-iteration 1x1 "lane absorber" matmuls read one freshly-DMA'd tile each
    (and write a persistent scratch PSUM tile), so each carries exactly one
    DMA-lane wait and advances the PE's observed clock;
  - PSUM tiles are allocated once and reused manually (no pool recycling), so
    no TileRelease edges exist on PSUM: the first matmul of a group carries
    only the WAR wait on the previous rotation's PSUM->SBUF copy.
"""

import numpy as np
import ml_dtypes

import concourse.bass as bass
import concourse.mybir as mybir
from concourse.tile import TileContext
from concourse.bass_utils import run_bass_kernel_spmd

B_SZ, IN_DIM, HID = 262144, 128, 256
N_CORES = 8
S = B_SZ // N_CORES     # 32768 rows per core
P = 128
NQ = HID // 64          # 4 interleaved chunks of 64 complex dims
COLS = 2048             # batch columns per outer iteration
OUTER = S // COLS       # 16
MMF = 512               # matmul free dim (one fp32 PSUM bank)
NBLK = COLS // MMF      # 4

# consts_w (128, 512) bf16: [:, q*128 : q*128+64] = w_re[:, 64q:64q+64],
#                           [:, q*128+64 : (q+1)*128] = w_im[:, 64q:64q+64]
# consts_h (128, 512) fp8e4m3: [:, q*128 : (q+1)*128] = Wh_q (Lambda block)
WCOLS = 512

F32 = mybir.dt.float32
BF16 = mybir.dt.bfloat16
F8 = mybir.dt.float8e4
NPBF16 = ml_dtypes.bfloat16
NPF8 = ml_dtypes.float8_e4m3

_cache = {}

# Stashed BassKernelResults from the most recent run (for test harnesses).
LAST_RESULTS = None


def _build():
    if "nc" in _cache:
        return _cache["nc"]

    nc = bass.Bass(trn_type="TRN2")

    x_t = nc.dram_tensor("x_t", (P, S), BF16, kind="ExternalInput")
    h_t = nc.dram_tensor("h_t", (P, NQ * S), F8, kind="ExternalInput")
    consts_w = nc.dram_tensor("consts_w", (P, WCOLS), BF16, kind="ExternalInput")
    consts_h = nc.dram_tensor("consts_h", (P, WCOLS), F8, kind="ExternalInput")
    o_t = nc.dram_tensor("o_t", (P, NQ * S), BF16, kind="ExternalOutput")

    C4 = NQ * COLS  # one outer iteration's per-partition h/o slab

    with TileContext(nc) as tc:
        with (
            tc.tile_pool(name="cpool", bufs=1) as cpool,
            tc.tile_pool(name="xin", bufs=3) as xin,
            tc.tile_pool(name="hin", bufs=3) as hin,
            tc.tile_pool(name="outp", bufs=3) as outp,
            tc.tile_pool(name="psum", bufs=1, space="PSUM") as psum,
        ):
            csb = cpool.tile([P, WCOLS], BF16)
            nc.gpsimd.dma_start(csb[:], consts_w[:, :])
            csh = cpool.tile([P, WCOLS], F8)
            nc.gpsimd.dma_start(csh[:], consts_h[:, :])
            # 7 persistent data PSUM tiles + 1 scratch; allocated once so no
            # TileRelease/realloc wait sets ever form on PSUM.
            ps_tiles = [psum.tile([P, MMF], F32, tag=f"ps{i}", name=f"ps{i}")
                        for i in range(7)]
            scratch = psum.tile([P, 8], F32, tag="scratch")
            _cache["ps_idx"] = 0

            def lane_absorb(tile_ap):
                # 1x1 matmul reading the freshly-DMA'd tile: carries exactly
                # one DMA-lane wait, advancing the PE's observed clock so the
                # real matmuls don't re-wait on that lane.
                nc.tensor.matmul(scratch[0:1, 0:1], tile_ap, tile_ap,
                                 start=True, stop=True, skip_group_check=True)

            def w_q(q):
                return csb[:, q * P:(q + 1) * P]

            def wh_q(q):
                return csh[:, q * P:(q + 1) * P]

            lane_absorb(csb[0:1, 0:1])
            lane_absorb(csh[0:1, 0:1])

            for o in range(OUTER):
                xt = xin.tile([P, COLS], BF16)
                nc.gpsimd.dma_start(xt[:], x_t[:, o * COLS:(o + 1) * COLS])
                hh = hin.tile([P, C4], F8)
                # h loads ride the Sync HWDGE ring; x + stores use other
                # rings so the three DGE paths spread descriptor pressure.
                nc.sync.dma_start(hh[:], h_t[:, o * C4:(o + 1) * C4])
                lane_absorb(xt[0:1, 0:1])
                lane_absorb(hh[0:1, 0:1])

                oo = outp.tile([P, C4], BF16)

                for q in range(NQ):
                    for b in range(NBLK):
                        bs = slice(b * MMF, (b + 1) * MMF)
                        qs = slice(q * COLS + b * MMF, q * COLS + (b + 1) * MMF)
                        ps = ps_tiles[_cache["ps_idx"] % 7]
                        _cache["ps_idx"] += 1
                        nc.tensor.matmul(ps[:], w_q(q), xt[:, bs],
                                         start=True, stop=False)
                        nc.tensor.matmul(ps[:], wh_q(q), hh[:, qs],
                                         start=False, stop=True)
                        # One engine per output half: q<2 <- ACT, q>=2 <- DVE.
                        if q < 2:
                            nc.scalar.copy(oo[:, qs], ps[:])
                        else:
                            nc.vector.tensor_copy(oo[:, qs], ps[:])

                half = 2 * COLS
                # ACT half stored by ACT itself (program order -> no sem
                # wait, own HWDGE ring); DVE half via gpsimd SWDGE (DVE
                # cannot issue DMAs) with the usual single-sem wait.
                nc.scalar.dma_start(o_t[:, o * C4: o * C4 + half],
                                    oo[:, 0:half])
                nc.gpsimd.dma_start(o_t[:, o * C4 + half:(o + 1) * C4],
                                    oo[:, half:C4])

    _split_multiwaits(nc)
    _cache["nc"] = nc
    return nc


def _split_multiwaits(nc):
    """walrus codegen allows exactly one semaphore wait per instruction.
    Move all-but-one wait of every multi-wait instruction onto single-wait
    NOP instructions spliced immediately before it on the same engine
    (engines execute their stream in order, so semantics are unchanged)."""
    k = 0
    for bb in nc.m.functions[0].blocks:
        new_list = []
        for ins in bb.instructions:
            si = ins.sync_info
            if si is not None and si.on_wait and len(si.on_wait) > 1:
                for w in si.on_wait[:-1]:
                    nop = mybir.InstNoOp(
                        name=f"WN-{k}", engine=ins.engine,
                        sync_info=mybir.SyncInfo(on_wait=[w], on_update=[]),
                    )
                    k += 1
                    new_list.append(nop)
                si.on_wait = [si.on_wait[-1]]
            new_list.append(ins)
        bb.instructions[:] = new_list


def kernel(inputs, h_re, h_im, nu_log, theta_log, B_real, B_img, gamma_log):
    global LAST_RESULTS
    inputs = np.asarray(inputs, dtype=np.float32)
    h_re = np.asarray(h_re, dtype=np.float32)
    h_im = np.asarray(h_im, dtype=np.float32)
    nu_log = np.asarray(nu_log, dtype=np.float32)
    theta_log = np.asarray(theta_log, dtype=np.float32)
    B_real = np.asarray(B_real, dtype=np.float32)
    B_img = np.asarray(B_img, dtype=np.float32)
    gamma_log = np.asarray(gamma_log, dtype=np.float32)

    # Tiny parameter math on host (matches the f32 reference computation).
    mag = np.exp(-np.exp(nu_log))          # (1, H)
    theta = np.exp(theta_log)              # (1, H)
    lam_re = (mag * np.cos(theta))[0]      # (H,)
    lam_im = (mag * np.sin(theta))[0]      # (H,)
    scale = np.exp(gamma_log).T            # (H, 1)
    w_re = (scale * B_real).T              # (IN_DIM, H)
    w_im = (scale * B_img).T               # (IN_DIM, H)

    consts_w = np.zeros((P, WCOLS), np.float32)
    consts_h = np.zeros((P, WCOLS), np.float32)
    u = np.arange(64)
    for q in range(NQ):
        consts_w[:, q * P: q * P + 64] = w_re[:, 64 * q: 64 * q + 64]
        consts_w[:, q * P + 64: (q + 1) * P] = w_im[:, 64 * q: 64 * q + 64]
        lrq = lam_re[64 * q: 64 * q + 64]
        liq = lam_im[64 * q: 64 * q + 64]
        consts_h[2 * u, q * P + u] = lrq
        consts_h[2 * u + 1, q * P + u] = -liq
        consts_h[2 * u, q * P + 64 + u] = liq
        consts_h[2 * u + 1, q * P + 64 + u] = lrq
    consts_w = consts_w.astype(NPBF16)
    consts_h = consts_h.astype(NPF8)

    in_maps = []
    for core in range(N_CORES):
        sl = slice(core * S, (core + 1) * S)
        # h2[p, o, q, j]: even partition 2u = h_re[o*COLS+j, 64q+u],
        #                 odd  partition 2u+1 = h_im[o*COLS+j, 64q+u]
        hc = np.empty((P, OUTER, NQ, COLS), NPF8)
        hc[0::2] = h_re[sl].reshape(OUTER, COLS, NQ, 64).transpose(3, 0, 2, 1)
        hc[1::2] = h_im[sl].reshape(OUTER, COLS, NQ, 64).transpose(3, 0, 2, 1)
        in_maps.append({
            "x_t": inputs[sl].T.astype(NPBF16),
            "h_t": hc.reshape(P, NQ * S),
            "consts_w": consts_w,
            "consts_h": consts_h,
        })

    nc = _build()
    res = run_bass_kernel_spmd(nc, in_maps, core_ids=list(range(N_CORES)))
    LAST_RESULTS = res

    out = np.empty((2, B_SZ, HID), np.float32)
    for core in range(N_CORES):
        sl = slice(core * S, (core + 1) * S)
        # o_t[p, o, q, j]: p<64 -> out_re[o*COLS+j, 64q+p],
        #                  p>=64 -> out_im[o*COLS+j, 64q+p-64]
        oc = res.results[core]["o_t"].reshape(P, OUTER, NQ, COLS)
        oc = oc.astype(np.float32)
        out[0, sl] = oc[:64].transpose(1, 3, 2, 0).reshape(S, HID)
        out[1, sl] = oc[64:].transpose(1, 3, 2, 0).reshape(S, HID)
    return out
